# revision 3
# baseline (speedup 1.0000x reference)
"""Multi-head attention Bass/Tile kernel for Trainium2.

Full inputs: q,k,v [8, 16, 1024, 128] fp32. Shards batch across 8 cores.
Per core/head: scores^T = (K @ Q^T)/128 via PE (bf16), exp on ACT (scale
fused), PV with P^T stationary and V||ones moving so the softmax
denominator falls out of the same matmul pass; normalize on DVE.
"""

import os
from contextlib import ExitStack

import numpy as np

import concourse.bass as bass
import concourse.tile as tile
from concourse import bacc, mybir
from concourse.bass_utils import run_bass_kernel_spmd
from concourse.masks import make_identity

H, S, D = 16, 1024, 128
NB = S // 128  # 8 blocks of 128 along sequence
FP32 = mybir.dt.float32
BF16 = mybir.dt.bfloat16
AF = mybir.ActivationFunctionType


def _emit_head_front(nc, tc, pools, aps, h):
    """Loads + transposes + QK^T + exp for head h. Returns (ptiles, va)."""
    (ld_pool, tq_pool, v_pool, pt_pool, _out_pool, _small_pool,
     ps_t, ps_s, _ps_o, ident) = pools
    q, k, v, _out = aps

    # Natural-layout loads with fp32->bf16 cast during DMA (SWDGE).
    qn = ld_pool.tile([128, S], BF16, tag="qn")
    kn = ld_pool.tile([128, S], BF16, tag="kn")
    nc.gpsimd.dma_start(
        out=qn[:].rearrange("p (sb d) -> p sb d", d=D),
        in_=q[h].rearrange("(sb p) d -> p sb d", p=128),
    )
    nc.gpsimd.dma_start(
        out=kn[:].rearrange("p (sb d) -> p sb d", d=D),
        in_=k[h].rearrange("(sb p) d -> p sb d", p=128),
    )

    # V augmented with a ones column: [128, NB*(D+1)] bf16.
    va = v_pool.tile([128, NB * (D + 1)], BF16, tag="va")
    va3 = va[:].rearrange("p (ib e) -> p ib e", e=D + 1)
    nc.gpsimd.dma_start(
        out=va3[:, :, 0:D],
        in_=v[h].rearrange("(ib p) d -> p ib d", p=128),
    )
    nc.vector.memset(va3[:, :, D : D + 1], 1.0)

    # PE transposes: qT,kT [d, s].
    qT = tq_pool.tile([128, S], BF16, tag="qT")
    kT = tq_pool.tile([128, S], BF16, tag="kT")
    for src, dst in ((qn, qT), (kn, kT)):
        for sb in range(NB):
            pt = ps_t.tile([128, 128], BF16)
            nc.tensor.transpose(pt[:], src[:, sb * 128 : (sb + 1) * 128], ident[:])
            nc.vector.tensor_copy(dst[:, sb * 128 : (sb + 1) * 128], pt[:])

    # QK^T (transposed scores) + exp. ptiles[ib][i, j] = exp(s[j, i]/128).
    ptiles = []
    for ib in range(NB):
        ps = ps_s.tile([128, S], FP32)
        for jh in range(2):
            nc.tensor.matmul(
                ps[:, jh * 512 : (jh + 1) * 512],
                kT[:, ib * 128 : (ib + 1) * 128],
                qT[:, jh * 512 : (jh + 1) * 512],
                start=True,
                stop=True,
            )
        ptile = pt_pool.tile([128, S], BF16, tag=f"pt{ib}")
        nc.scalar.activation(ptile[:], ps[:], AF.Exp, scale=1.0 / D)
        ptiles.append(ptile)
    return ptiles, va


def _emit_head_back(nc, tc, pools, aps, h, ptiles, va):
    """PV + normalize + store for head h."""
    (_ld, _tq, _v, _pt, out_pool, small_pool, _ps_t, _ps_s, ps_o, _ident) = pools
    _q, _k, _v, out = aps

    va3 = va[:].rearrange("p (ib e) -> p ib e", e=D + 1)
    ob = out_pool.tile([128, S], FP32, tag="ob")
    for jb in range(NB):
        po = ps_o.tile([128, D + 1], FP32)
        for ib in range(NB):
            nc.tensor.matmul(
                po[:],
                ptiles[ib][:, jb * 128 : (jb + 1) * 128],
                va3[:, ib, :],
                start=(ib == 0),
                stop=(ib == NB - 1),
            )
        rec = small_pool.tile([128, 1], FP32, tag="rec")
        nc.vector.reciprocal(rec[:], po[:, D : D + 1])
        nc.vector.tensor_scalar_mul(
            ob[:, jb * 128 : (jb + 1) * 128], po[:, 0:D], rec[:]
        )
    nc.sync.dma_start(
        out=out[h].rearrange("(jb p) d -> p jb d", p=128),
        in_=ob[:].rearrange("p (jb d) -> p jb d", d=D),
    )


def build_bass():
    nc = bacc.Bacc("TRN2", target_bir_lowering=False, debug=False)
    q = nc.dram_tensor("q", [H, S, D], FP32, kind="ExternalInput").ap()
    k = nc.dram_tensor("k", [H, S, D], FP32, kind="ExternalInput").ap()
    v = nc.dram_tensor("v", [H, S, D], FP32, kind="ExternalInput").ap()
    out = nc.dram_tensor("out", [H, S, D], FP32, kind="ExternalOutput").ap()
    aps = (q, k, v, out)

    with ExitStack() as ctx:
        tc = ctx.enter_context(tile.TileContext(nc))
        const_pool = ctx.enter_context(tc.tile_pool(name="const", bufs=1))
        ident = const_pool.tile([128, 128], BF16)
        make_identity(nc, ident[:])

        ld_pool = ctx.enter_context(tc.tile_pool(name="loads", bufs=2))
        tq_pool = ctx.enter_context(tc.tile_pool(name="qkT", bufs=2))
        v_pool = ctx.enter_context(tc.tile_pool(name="vaug", bufs=2))
        pt_pool = ctx.enter_context(tc.tile_pool(name="pT", bufs=2))
        out_pool = ctx.enter_context(tc.tile_pool(name="outs", bufs=2))
        small_pool = ctx.enter_context(tc.tile_pool(name="small", bufs=4))
        ps_t = ctx.enter_context(tc.tile_pool(name="ps_t", bufs=2, space="PSUM"))
        ps_s = ctx.enter_context(tc.tile_pool(name="ps_s", bufs=2, space="PSUM"))
        ps_o = ctx.enter_context(tc.tile_pool(name="ps_o", bufs=2, space="PSUM"))
        pools = (ld_pool, tq_pool, v_pool, pt_pool, out_pool, small_pool,
                 ps_t, ps_s, ps_o, ident)

        # Software pipeline: head h front (loads/QK/exp) overlaps head h-1
        # back (PV/store).
        prev = None
        for h in range(H):
            cur = _emit_head_front(nc, tc, pools, aps, h)
            if prev is not None:
                _emit_head_back(nc, tc, pools, aps, h - 1, *prev)
            prev = cur
        _emit_head_back(nc, tc, pools, aps, H - 1, *prev)
    nc.finalize()
    return nc


_NC_CACHE = None


def _get_nc():
    global _NC_CACHE
    if _NC_CACHE is None:
        _NC_CACHE = build_bass()
    return _NC_CACHE


def run_sharded(q, k, v, **kwargs):
    """q,k,v: full [8, 16, 1024, 128] fp32. Returns (results, BassKernelResults)."""
    B = q.shape[0]
    nc = _get_nc()
    in_maps = [
        {
            "q": np.ascontiguousarray(q[c], dtype=np.float32),
            "k": np.ascontiguousarray(k[c], dtype=np.float32),
            "v": np.ascontiguousarray(v[c], dtype=np.float32),
        }
        for c in range(B)
    ]
    res = run_bass_kernel_spmd(nc, in_maps, core_ids=list(range(B)), **kwargs)
    out = np.stack([res.results[c]["out"] for c in range(B)]).astype(np.float32)
    return out, res


def kernel(q, k, v):
    q = np.asarray(q)
    k = np.asarray(k)
    v = np.asarray(v)
    out, _ = run_sharded(q, k, v)
    return out


if __name__ == "__main__":
    rng = np.random.default_rng(0)
    q = rng.standard_normal((8, H, S, D), dtype=np.float32)
    k = rng.standard_normal((8, H, S, D), dtype=np.float32)
    v = rng.standard_normal((8, H, S, D), dtype=np.float32)
    o = kernel(q, k, v)
    print("out", o.shape, o.dtype, float(np.abs(o).mean()))
